# revision 1
# baseline (speedup 1.0000x reference)
"""Causal single-head attention on 8 trn2 NeuronCores.

Problem (hardcoded): x [256,256,384] f32, Wq/Wk/Wv [384,64] f32
  q,k,v = x@W;  S = q@k^T * 384**-0.5; causal softmax; out = P@v  [256,256,64]

Sharding: data-parallel over batch B=256 -> 32 batches per core; weights
replicated. Per batch (T=256 tokens, C=384, H=64), per core:

  1. DMA x_b [256,384] as two [128,384] tiles (t-chunks).
  2. PE-transpose (fp32, exact) 6 128x128 blocks -> x^T [384c, 256t] in SBUF
     (rounded to f32r by the PSUM->SBUF evacuation copies).
  3. kT/qT = Wk^T@x^T, Wq^T@x^T   [64,256] each (f32r matmuls, N=256)
     vT = Wv^T@x^T [64,256]; append ones row -> v'T [65,256]; PE-transpose to
     v' [128,65] per t-chunk (v natural + ones column).
  4. S^T[j,i] per j-chunk: lhsT=kT chunk, rhs=qT  -> [128,256] PSUM.
     P^T = exp(scale*S^T) via ACT (PSUM->SBUF, f32r), then causal mask:
     multiplicative 0/1 upper-triangular 128x128 tile (+ zeroing the
     all-masked left half of chunk 1). No max-subtraction: |scale*S| <~ 3.
  5. O'^T [65,256] = sum_j v'[j,:]^T... accumulated over both j-chunks.
     Row 64 = softmax denominators (ones row of v').
  6. PE-transpose O'^T back to [128,65] per t-chunk; normalize cols 0:64 by
     reciprocal of col 64; DMA out.
"""
import numpy as np

N_CORES = 8
B, T, C, H = 256, 256, 384, 64
NB = B // N_CORES          # 32 batches per core
SCALE = float(C) ** -0.5

_state = {}


def _build():
    import concourse.bacc as bacc
    import concourse.tile as tile
    import concourse.mybir as mybir
    from concourse.masks import make_identity, make_upper_triangular

    dt = mybir.dt
    f32 = dt.float32
    f32r = dt.float32r
    AF = mybir.ActivationFunctionType

    nc = bacc.Bacc("TRN2", target_bir_lowering=False)
    x_d = nc.dram_tensor("x", [NB, T, C], f32, kind="ExternalInput")
    wq_d = nc.dram_tensor("Wq", [C, H], f32, kind="ExternalInput")
    wk_d = nc.dram_tensor("Wk", [C, H], f32, kind="ExternalInput")
    wv_d = nc.dram_tensor("Wv", [C, H], f32, kind="ExternalInput")
    out_d = nc.dram_tensor("out", [NB, T, H], f32, kind="ExternalOutput")

    with tile.TileContext(nc) as tc:
        with tc.tile_pool(name="setup", bufs=1) as setup, \
             tc.tile_pool(name="xin", bufs=3) as xin, \
             tc.tile_pool(name="work", bufs=3) as work, \
             tc.tile_pool(name="ps", bufs=1, space="PSUM") as ps:

            # --- one-time setup ---
            ident = setup.tile([128, 128], f32)
            make_identity(nc, ident)
            mask_st = setup.tile([128, 128], f32)
            make_upper_triangular(nc, mask_st, val=1.0, diag=True)
            mask = setup.tile([128, 128], f32r)
            nc.vector.tensor_copy(mask, mask_st)

            w_stage = setup.tile([128, 3 * C // 128 * 0 + 576], f32)  # [128, 576]
            # cc-chunk cc occupies cols cc*192:(cc+1)*192 as [Wk|Wq|Wv]
            for cc in range(3):
                nc.sync.dma_start(out=w_stage[:, cc * 192 + 0: cc * 192 + 64],
                                  in_=wk_d[cc * 128:(cc + 1) * 128, :])
                nc.sync.dma_start(out=w_stage[:, cc * 192 + 64: cc * 192 + 128],
                                  in_=wq_d[cc * 128:(cc + 1) * 128, :])
                nc.sync.dma_start(out=w_stage[:, cc * 192 + 128: cc * 192 + 192],
                                  in_=wv_d[cc * 128:(cc + 1) * 128, :])
            w_all = setup.tile([128, 576], f32r)
            nc.vector.tensor_copy(w_all, w_stage)  # round to f32r

            def wslice(cc, which):  # which: 0=k 1=q 2=v
                lo = cc * 192 + which * 64
                return w_all[:, lo:lo + 64]

            # --- per-batch pipeline ---
            for b in range(NB):
                x0 = xin.tile([128, C], f32)
                x1 = xin.tile([128, C], f32)
                nc.sync.dma_start(out=x0, in_=x_d[b, 0:128, :])
                nc.sync.dma_start(out=x1, in_=x_d[b, 128:256, :])

                # transpose x -> x^T  (xtps_a holds cc0+cc1, xtps_b holds cc2)
                xtps_a = ps.tile([128, 512], f32)
                xtps_b = ps.tile([128, 256], f32)
                for cc in range(3):
                    dst = xtps_a if cc < 2 else xtps_b
                    base = (cc % 2) * 256 if cc < 2 else 0
                    nc.tensor.transpose(dst[:, base:base + 128],
                                        x0[:, cc * 128:(cc + 1) * 128], ident)
                    nc.tensor.transpose(dst[:, base + 128:base + 256],
                                        x1[:, cc * 128:(cc + 1) * 128], ident)
                xt = work.tile([128, 768], f32r)
                nc.scalar.copy(xt[:, 0:512], xtps_a)
                nc.vector.tensor_copy(xt[:, 512:768], xtps_b)

                def xts(cc):
                    return xt[:, cc * 256:(cc + 1) * 256]

                # kT / qT  -> one PSUM bank [64, 512]
                kqps = ps.tile([64, 512], f32)
                for cc in range(3):
                    nc.tensor.matmul(kqps[:, 0:256], wslice(cc, 0), xts(cc),
                                     start=(cc == 0), stop=(cc == 2))
                for cc in range(3):
                    nc.tensor.matmul(kqps[:, 256:512], wslice(cc, 1), xts(cc),
                                     start=(cc == 0), stop=(cc == 2))
                kq_k = work.tile([64, 256], f32r)
                kq_q = work.tile([64, 256], f32r)
                nc.vector.tensor_copy(kq_k, kqps[:, 0:256])
                nc.scalar.copy(kq_q, kqps[:, 256:512])

                # vT [64,256] -> v'T [65,256] (ones row) -> v' [128,65] per tc
                vtps = ps.tile([64, 256], f32)
                for cc in range(3):
                    nc.tensor.matmul(vtps, wslice(cc, 2), xts(cc),
                                     start=(cc == 0), stop=(cc == 2))
                vtp = work.tile([65, 256], f32)
                nc.scalar.copy(vtp[0:64, :], vtps)
                nc.gpsimd.memset(vtp[64:65, :], 1.0)
                vpps = ps.tile([128, 130], f32)
                vp = work.tile([128, 130], f32r)
                nc.tensor.transpose(vpps[:, 0:65], vtp[:, 0:128],
                                    ident[0:65, 0:65])
                nc.tensor.transpose(vpps[:, 65:130], vtp[:, 128:256],
                                    ident[0:65, 0:65])
                nc.vector.tensor_copy(vp, vpps)
                vp0 = vp[:, 0:65]
                vp1 = vp[:, 65:130]

                # S^T per j-chunk + exp + causal mask
                stps = ps.tile([128, 512], f32)
                nc.tensor.matmul(stps[:, 0:256], kq_k[:, 0:128], kq_q,
                                 start=True, stop=True)
                nc.tensor.matmul(stps[:, 256:512], kq_k[:, 128:256], kq_q,
                                 start=True, stop=True)
                pt0 = work.tile([128, 256], f32r)
                pt1 = work.tile([128, 128], f32r)
                nc.scalar.activation(pt0, stps[:, 0:256], AF.Exp, scale=SCALE)
                # chunk-1 rows attend only to keys j>=128 -> cols 128:256
                nc.scalar.activation(pt1, stps[:, 384:512], AF.Exp, scale=SCALE)
                nc.vector.tensor_mul(pt0[:, 0:128], pt0[:, 0:128], mask)
                nc.vector.tensor_mul(pt1, pt1, mask)

                # O'^T [65,256] accumulate over j-chunks (chunk 1 only touches
                # output cols 128:256; cols 0:128 get no chunk-1 contribution)
                ops = ps.tile([65, 256], f32)
                nc.tensor.matmul(ops, vp0, pt0, start=True, stop=False)
                nc.tensor.matmul(ops[:, 128:256], vp1, pt1,
                                 start=False, stop=True)
                ot = work.tile([65, 256], f32)
                nc.vector.tensor_copy(ot, ops)

                # transpose back, normalize, store
                ofps = ps.tile([128, 130], f32)
                nc.tensor.transpose(ofps[:, 0:65], ot[:, 0:128],
                                    ident[0:65, 0:65])
                nc.tensor.transpose(ofps[:, 65:130], ot[:, 128:256],
                                    ident[0:65, 0:65])
                rec0 = work.tile([128, 1], f32)
                rec1 = work.tile([128, 1], f32)
                nc.vector.reciprocal(rec0, ofps[:, 64:65])
                nc.vector.reciprocal(rec1, ofps[:, 129:130])
                oo0 = work.tile([128, 64], f32)
                oo1 = work.tile([128, 64], f32)
                nc.vector.tensor_scalar_mul(oo0, ofps[:, 0:64], rec0)
                nc.scalar.mul(oo1, ofps[:, 65:129], rec1)
                nc.sync.dma_start(out=out_d[b, 0:128, :], in_=oo0)
                nc.sync.dma_start(out=out_d[b, 128:256, :], in_=oo1)

    nc.finalize()
    return nc


def kernel(x, Wq, Wk, Wv, _trace=False):
    from concourse.bass_utils import run_bass_kernel_spmd

    if "nc" not in _state:
        _state["nc"] = _build()
    nc = _state["nc"]

    x = np.ascontiguousarray(np.asarray(x, dtype=np.float32))
    wq = np.ascontiguousarray(np.asarray(Wq, dtype=np.float32))
    wk = np.ascontiguousarray(np.asarray(Wk, dtype=np.float32))
    wv = np.ascontiguousarray(np.asarray(Wv, dtype=np.float32))

    in_maps = [
        {"x": x[i * NB:(i + 1) * NB], "Wq": wq, "Wk": wk, "Wv": wv}
        for i in range(N_CORES)
    ]
    res = run_bass_kernel_spmd(nc, in_maps, core_ids=list(range(N_CORES)),
                               trace=_trace)
    _state["exec_time_ns"] = res.exec_time_ns
    _state["trace"] = res.instructions_and_trace
    return np.concatenate([res.results[i]["out"] for i in range(N_CORES)],
                          axis=0)



# revision 2
# speedup vs baseline: 1.5991x; 1.5991x over previous
"""Causal single-head attention on 8 trn2 NeuronCores (cost-model optimized).

Problem (hardcoded): x [256,256,384] f32, Wq/Wk/Wv [384,64] f32
  q,k,v = x@W;  S = q@k^T * 384**-0.5; causal softmax; out = P@v  [256,256,64]

Sharding: data-parallel over batch B=256 -> 32 batches per core; weights
replicated.  All on-chip compute in bf16 (rel-err budget 2e-2).

Per core, batches processed in pairs (16 iterations), per batch b:
  1. x loaded 4-batches-per-DMA via SWDGE cast f32->bf16 (halves modeled
     DMA time; charged on output bytes).
  2. PE-transposes x (bf16 identity => 1 cycle/row) -> x^T in PSUM (bf16)
     -> DVE evac (2x mode) to SBUF.
  3. kqT = [Wk|Wq]^T @ x^T  (3 matmuls N=512 covering both batches of the
     pair) -> PSUM f32 -> ACT evac -> kq_sb [128,2,256] (kT rows 0:64,
     qT rows 64:128).
  4. qT relocated to partition base 0 via SBUF->SBUF DMA (matmul operands
     must share their SB start partition).
  5. v natural [128t,64] per t-chunk: lhsT = x^T blocks, rhs = Wv chunk
     (N=64; matmul cost is out-free-size only) -> v' [128,2,65] with a
     memset ones column (softmax denominator trick).
  6. S^T per key-chunk j: lhsT = kT[:,j], rhs = qT (N=256 / N=128 for the
     diag-only part).  exp via one ACT activation (scale folded in),
     causal diag blocks masked by DVE multiply with a triangular tile.
  7. O natural: lhsT = P^T blocks (stationary), rhs = v' (N=65) -> PSUM
     [128, 65]; col 64 = softmax denominator.  DVE reciprocal + per-chunk
     scalar multiply normalizes straight into the bf16 output tile.
  8. Output stored 4-batches-per-DMA as bf16; host converts to f32.
"""
import numpy as np

N_CORES = 8
B, T, C, H = 256, 256, 384, 64
NB = B // N_CORES          # 32 batches per core
SCALE = float(C) ** -0.5
NEG = -30.0                # pre-exp additive mask value (exp(-30+4) ~ 5e-12)

_state = {}


def _build():
    import concourse.bacc as bacc
    import concourse.tile as tile
    import concourse.mybir as mybir
    from concourse.masks import make_identity, make_upper_triangular

    dt = mybir.dt
    f32 = dt.float32
    bf16 = dt.bfloat16
    AF = mybir.ActivationFunctionType

    nc = bacc.Bacc("TRN2", target_bir_lowering=False)
    # x pre-split into t-chunks: [NB, 2, 128, 384]; same bytes as [NB,256,384]
    x_d = nc.dram_tensor("x", [NB, 2, 128, C], f32, kind="ExternalInput")
    wq_d = nc.dram_tensor("Wq", [3, 128, H], f32, kind="ExternalInput")
    wk_d = nc.dram_tensor("Wk", [3, 128, H], f32, kind="ExternalInput")
    wv_d = nc.dram_tensor("Wv", [3, 128, H], f32, kind="ExternalInput")
    out_d = nc.dram_tensor("out", [NB, 2, 128, H], bf16, kind="ExternalOutput")

    with tile.TileContext(nc) as tc:
        with tc.tile_pool(name="setup", bufs=1) as setup, \
             tc.tile_pool(name="xin", bufs=2) as xin, \
             tc.tile_pool(name="xts_p", bufs=2) as xts_p, \
             tc.tile_pool(name="sb", bufs=2) as sb, \
             tc.tile_pool(name="oo_p", bufs=2) as oo_p, \
             tc.tile_pool(name="ps_xt", bufs=2, space="PSUM") as ps_xt, \
             tc.tile_pool(name="ps_kq", bufs=1, space="PSUM") as ps_kq, \
             tc.tile_pool(name="ps_v", bufs=1, space="PSUM") as ps_v, \
             tc.tile_pool(name="ps_s", bufs=1, space="PSUM") as ps_s, \
             tc.tile_pool(name="ps_o", bufs=2, space="PSUM") as ps_o:

            # ---------------- one-time setup ----------------
            ident = setup.tile([128, 128], bf16)
            make_identity(nc, ident)
            mask_st = setup.tile([128, 128], f32)
            make_upper_triangular(nc, mask_st, val=1.0, diag=True)
            mask = setup.tile([128, 128], bf16)
            nc.vector.tensor_copy(mask, mask_st)

            # weights, cast to bf16 during DMA (SWDGE)
            wkq = setup.tile([128, 3, 128], bf16)   # [Wk | Wq] per C-chunk
            nc.gpsimd.dma_start(out=wkq[:, :, 0:H],
                                in_=wk_d[:, :, :].transpose([1, 0, 2]))
            nc.gpsimd.dma_start(out=wkq[:, :, H:128],
                                in_=wq_d[:, :, :].transpose([1, 0, 2]))
            wv = setup.tile([128, 3, H], bf16)
            nc.gpsimd.dma_start(out=wv, in_=wv_d[:, :, :].transpose([1, 0, 2]))

            # ---------------- main loop: 16 iterations x 2 batches ----------
            x4 = None
            oo4 = None
            for it in range(NB // 2):
                if it % 2 == 0:
                    g = it // 2  # 4-batch group index
                    x4 = xin.tile([128, 4, 2, C], bf16, tag="x4")
                    nc.gpsimd.dma_start(
                        out=x4,
                        in_=x_d[4 * g:4 * g + 4, :, :, :].transpose([2, 0, 1, 3]))
                    oo4 = oo_p.tile([128, 4, 2, H], bf16, tag="oo4")

                # -- transpose x -> x^T (bf16 PSUM) and evacuate ------------
                xts = xts_p.tile([128, 2, 3, 256], bf16, tag="xts")
                for b in range(2):
                    bb = (it % 2) * 2 + b
                    xtp = ps_xt.tile([128, 768], bf16, tag="xtp")
                    for cc in range(3):
                        for tc_ in range(2):
                            nc.tensor.transpose(
                                xtp[:, cc * 256 + tc_ * 128:cc * 256 + tc_ * 128 + 128],
                                x4[:, bb, tc_, cc * 128:cc * 128 + 128],
                                ident)
                    nc.vector.tensor_copy(xts[:, b, :, :], xtp)

                # -- kq projection: kT rows 0:64, qT rows 64:128 ------------
                kq_ps = ps_kq.tile([128, 512], f32, tag="kq")  # (b, t) flat
                for cc in range(3):
                    nc.tensor.matmul(kq_ps, wkq[:, cc, :], xts[:, :, cc, :],
                                     start=(cc == 0), stop=(cc == 2))
                kq_sb = sb.tile([128, 2, 256], bf16, tag="kq_sb")
                nc.scalar.copy(kq_sb[:, :, :], kq_ps)
                # relocate qT to partition base 0 (SBUF->SBUF DMA)
                qt_sb = sb.tile([64, 2, 256], bf16, tag="qt_sb")
                nc.sync.dma_start(out=qt_sb, in_=kq_sb[64:128, :, :])

                # -- v projection (natural layout) + ones column ------------
                v_ps = ps_v.tile([128, 4, H], f32, tag="v")
                for b in range(2):
                    for tc_ in range(2):
                        for cc in range(3):
                            nc.tensor.matmul(
                                v_ps[:, 2 * b + tc_, :],
                                xts[:, b, cc, tc_ * 128:tc_ * 128 + 128],
                                wv[:, cc, :],
                                start=(cc == 0), stop=(cc == 2))
                vp = sb.tile([128, 2, 2, 65], bf16, tag="vp")
                nc.vector.tensor_copy(vp[:, :, :, 0:H], v_ps)
                nc.gpsimd.memset(vp[:, :, :, H:H + 1], 1.0)

                # -- S^T + exp + causal mask --------------------------------
                stps = ps_s.tile([128, 2, 512], f32, tag="st")
                for b in range(2):
                    nc.tensor.matmul(stps[:, b, 0:256],
                                     kq_sb[0:64, b, 0:128], qt_sb[:, b, :],
                                     start=True, stop=True)
                    nc.tensor.matmul(stps[:, b, 256:384],
                                     kq_sb[0:64, b, 128:256],
                                     qt_sb[:, b, 128:256],
                                     start=True, stop=True)
                pt = sb.tile([128, 2, 3, 128], bf16, tag="pt")
                nc.scalar.activation(pt, stps[:, :, 0:384], AF.Exp, scale=SCALE)
                for b in range(2):
                    nc.vector.tensor_mul(pt[:, b, 0, :], pt[:, b, 0, :], mask)
                    nc.vector.tensor_mul(pt[:, b, 2, :], pt[:, b, 2, :], mask)

                # -- O natural (N=65; col 64 = softmax denominator) ---------
                o_ps = ps_o.tile([128, 4, 65], f32, tag="o")
                for b in range(2):
                    nc.tensor.matmul(o_ps[:, 2 * b, :],
                                     pt[:, b, 0, :], vp[:, b, 0, :],
                                     start=True, stop=True)
                    nc.tensor.matmul(o_ps[:, 2 * b + 1, :],
                                     pt[:, b, 1, :], vp[:, b, 0, :],
                                     start=True, stop=False)
                    nc.tensor.matmul(o_ps[:, 2 * b + 1, :],
                                     pt[:, b, 2, :], vp[:, b, 1, :],
                                     start=False, stop=True)

                # -- normalize ----------------------------------------------
                rec = sb.tile([128, 4], f32, tag="rec")
                nc.vector.reciprocal(rec, o_ps[:, :, H:H + 1])
                for b in range(2):
                    bb = (it % 2) * 2 + b
                    nc.vector.tensor_scalar_mul(
                        oo4[:, bb, 0, :], o_ps[:, 2 * b, 0:H], rec[:, 2 * b:2 * b + 1])
                    nc.scalar.mul(
                        oo4[:, bb, 1, :], o_ps[:, 2 * b + 1, 0:H],
                        rec[:, 2 * b + 1:2 * b + 2])

                if it % 2 == 1:
                    g = it // 2
                    nc.sync.dma_start(
                        out=out_d[4 * g:4 * g + 4, :, :, :].transpose([2, 0, 1, 3]),
                        in_=oo4)

    nc.finalize()
    return nc


def kernel(x, Wq, Wk, Wv, _trace=False):
    from concourse.bass_utils import run_bass_kernel_spmd

    if "nc" not in _state:
        _state["nc"] = _build()
    nc = _state["nc"]

    x = np.ascontiguousarray(np.asarray(x, dtype=np.float32)).reshape(B, 2, 128, C)
    wq = np.ascontiguousarray(np.asarray(Wq, dtype=np.float32)).reshape(3, 128, H)
    wk = np.ascontiguousarray(np.asarray(Wk, dtype=np.float32)).reshape(3, 128, H)
    wv = np.ascontiguousarray(np.asarray(Wv, dtype=np.float32)).reshape(3, 128, H)

    in_maps = [
        {"x": x[i * NB:(i + 1) * NB], "Wq": wq, "Wk": wk, "Wv": wv}
        for i in range(N_CORES)
    ]
    res = run_bass_kernel_spmd(nc, in_maps, core_ids=list(range(N_CORES)),
                               trace=_trace)
    _state["exec_time_ns"] = res.exec_time_ns
    _state["trace"] = res.instructions_and_trace
    out = np.concatenate(
        [np.asarray(res.results[i]["out"]).astype(np.float32) for i in range(N_CORES)],
        axis=0)
    return out.reshape(B, T, H)
